# revision 1
# baseline (speedup 1.0000x reference)
"""Multi-head masked attention on 8 Trainium2 NeuronCores.

Sharding: data-parallel over batch (B=2 -> 2 groups of 4 cores),
tensor-parallel over heads within a group (16 heads -> 4 heads/core).
Each core computes q/k/v projections for its 4 heads (column-sharded),
causal flash-style attention in the transposed (S^T) domain, and a
row-sharded partial o-projection. The host sums the 4 partials per
batch element and adds the output bias.

Self-contained: hardcodes shapes B=2, T=2048, C=1024, H=16, Dh=64.
"""

import sys

sys.path.insert(0, "/opt/trn_rl_repo")

import numpy as np

import concourse.bass as bass
import concourse.tile as tile
import concourse.mybir as mybir
from concourse import bacc
from concourse.bass import ts, ds
from concourse.masks import make_identity, make_lower_triangular

F32 = mybir.dt.float32
F32R = mybir.dt.float32r
BF16 = mybir.dt.bfloat16
AF = mybir.ActivationFunctionType
ALU = mybir.AluOpType

B, T, C = 2, 2048, 1024
H, DH = 16, 64
HPC = 4            # heads per core
DQC = HPC * DH     # 256 projected dims per core
N_CORES = 8
NEG = -1.0e30


def build_program():
    nc = bacc.Bacc("TRN2", target_bir_lowering=False, debug=False)

    xb = nc.dram_tensor("xb", [T, C], F32, kind="ExternalInput")
    wq = nc.dram_tensor("wq", [C, DQC], F32R, kind="ExternalInput")
    wk = nc.dram_tensor("wk", [C, DQC], F32R, kind="ExternalInput")
    wv = nc.dram_tensor("wv", [C, DQC], F32R, kind="ExternalInput")
    wo = nc.dram_tensor("wo", [DQC, C], F32R, kind="ExternalInput")
    bq = nc.dram_tensor("bq", [DQC], F32, kind="ExternalInput")
    bk = nc.dram_tensor("bk", [DQC], F32, kind="ExternalInput")
    bv = nc.dram_tensor("bv", [DQC], F32, kind="ExternalInput")
    out = nc.dram_tensor("out", [T, C], F32, kind="ExternalOutput")

    TC = T // 128    # 16 t-chunks of 128
    CC = C // 128    # 8 c-chunks
    TJ = T // 512    # 4 t-chunks of 512
    scale = 1.0 / np.sqrt(DH)

    with tile.TileContext(nc) as tc:
        with (
            tc.tile_pool(name="persist", bufs=1) as pp,
            tc.tile_pool(name="ps_s", bufs=2, space="PSUM") as ps_s,
            tc.tile_pool(name="ps_pv", bufs=2, space="PSUM") as ps_pv,
            tc.tile_pool(name="ps_misc", bufs=2, space="PSUM") as ps_misc,
        ):
            # ---- persistent sbuf tensors -------------------------------
            qT = pp.tile([128, 2, T], F32R, tag="qT")   # [p, pair, t]
            kT = pp.tile([128, 2, T], F32R, tag="kT")
            vA = pp.tile([128, TC, HPC * (DH + 1)], F32R, tag="vA")
            yT = pp.tile([128, 2, T], F32R, tag="yT")
            wo_sb = pp.tile([128, 2, C], F32R, tag="wo")
            ident = pp.tile([128, 128], F32, tag="ident")
            bqs = pp.tile([128, 2], F32, tag="bqs")
            bks = pp.tile([128, 2], F32, tag="bks")
            bvs = pp.tile([128, DQC], F32, tag="bvs")

            # constants
            make_identity(nc, ident[:])
            # atrif[p, f] = NEG where f < p (mask s>t inside a diagonal block)
            atrif = pp.tile([128, 128], F32, tag="atrif")
            make_lower_triangular(nc, atrif[:], val=NEG, diag=False)
            # ones column of v_aug (memset can't write f32r; copy from f32)
            vA4 = vA[:].rearrange("p s (h d) -> p s h d", d=DH + 1)
            onesf = pp.tile([128, TC * HPC], F32, tag="onesf")
            nc.gpsimd.memset(onesf[:], 1.0)
            nc.vector.tensor_copy(
                vA4[:, :, :, DH : DH + 1],
                onesf[:].rearrange("p (s h o) -> p s h o", h=HPC, o=1),
            )

            # biases
            nc.sync.dma_start(bqs[:], bq.ap().rearrange("(k p) -> p k", p=128))
            nc.vector.tensor_scalar_mul(bqs[:], bqs[:], scale)
            nc.sync.dma_start(bks[:], bk.ap().rearrange("(k p) -> p k", p=128))
            nc.sync.dma_start(
                bvs[0:1, :], bv.ap().rearrange("(o n) -> o n", o=1)
            )
            nc.gpsimd.partition_broadcast(bvs[:], bvs[0:1, :])

            # ---- phase 0/P: x^T + projections (scoped pools) -----------
            with (
                tc.tile_pool(name="xw", bufs=1) as pw,
                tc.tile_pool(name="xin", bufs=3) as px,
            ):
                xT = pw.tile([128, CC, T], F32R, tag="xT")
                wq_sb = pw.tile([128, CC, DQC], F32R, tag="wq")
                wk_sb = pw.tile([128, CC, DQC], F32R, tag="wk")
                wv_sb = pw.tile([128, CC, DQC], F32R, tag="wv")

                # x^T via PE transposes, 4 blocks per psum bank
                for tch in range(TC):
                    x_tile = px.tile([128, C], F32, tag="x")
                    nc.sync.dma_start(x_tile[:], xb.ap()[ts(tch, 128), :])
                    for ccg in range(CC // 4):
                        pt = ps_misc.tile([128, 512], F32, tag="misc")
                        for q in range(4):
                            cc = 4 * ccg + q
                            nc.tensor.transpose(
                                pt[:, ts(q, 128)], x_tile[:, ts(cc, 128)], ident[:]
                            )
                        nc.any.tensor_copy(
                            xT[:, ds(4 * ccg, 4), ts(tch, 128)],
                            pt[:].rearrange("p (c t) -> p c t", t=128),
                        )

                # weight DMAs after x tiles so x loads go first
                nc.sync.dma_start(
                    wq_sb[:], wq.ap().rearrange("(c p) d -> p c d", p=128)
                )
                nc.sync.dma_start(
                    wk_sb[:], wk.ap().rearrange("(c p) d -> p c d", p=128)
                )
                nc.sync.dma_start(
                    wv_sb[:], wv.ap().rearrange("(c p) d -> p c d", p=128)
                )
                nc.sync.dma_start(
                    wo_sb[:], wo.ap().rearrange("(k p) n -> p k n", p=128)
                )

                # Q^T / K^T projections: [p=dq_local, pair, t]
                for hp in range(2):
                    for tj in range(TJ):
                        pq = ps_misc.tile([128, 512], F32, tag="misc")
                        for cc in range(CC):
                            nc.tensor.matmul(
                                pq[:],
                                wq_sb[:, cc, ts(hp, 128)],
                                xT[:, cc, ts(tj, 512)],
                                start=(cc == 0),
                                stop=(cc == CC - 1),
                            )
                        nc.vector.tensor_scalar(
                            qT[:, hp, ts(tj, 512)],
                            pq[:],
                            scale,
                            bqs[:, hp : hp + 1],
                            ALU.mult,
                            ALU.add,
                        )
                        pk = ps_misc.tile([128, 512], F32, tag="misc")
                        for cc in range(CC):
                            nc.tensor.matmul(
                                pk[:],
                                wk_sb[:, cc, ts(hp, 128)],
                                xT[:, cc, ts(tj, 512)],
                                start=(cc == 0),
                                stop=(cc == CC - 1),
                            )
                        nc.vector.tensor_scalar(
                            kT[:, hp, ts(tj, 512)],
                            pk[:],
                            bks[:, hp : hp + 1],
                            None,
                            ALU.add,
                        )

                # V projection (natural layout) + bias, into v_aug slots
                for sc in range(TC):
                    pv = ps_misc.tile([128, 512], F32, tag="misc")
                    for cc in range(CC):
                        nc.tensor.matmul(
                            pv[:, :DQC],
                            xT[:, cc, ts(sc, 128)],
                            wv_sb[:, cc, :],
                            start=(cc == 0),
                            stop=(cc == CC - 1),
                        )
                    nc.vector.tensor_tensor(
                        vA4[:, sc, :, :DH],
                        pv[:, :DQC].rearrange("p (h d) -> p h d", d=DH),
                        bvs[:].rearrange("p (h d) -> p h d", d=DH),
                        ALU.add,
                    )

            # ---- phase A: attention + o-projection ---------------------
            with (
                tc.tile_pool(name="psb", bufs=3) as pexp,
                tc.tile_pool(name="small", bufs=4) as psm,
                tc.tile_pool(name="outp", bufs=3) as pout,
            ):
                for tj in range(TJ):
                    n_sc = 4 * (tj + 1)
                    for hp in range(2):
                        hA, hB = 2 * hp, 2 * hp + 1
                        ppv_A = ps_pv.tile([128, 512], F32, tag="pv")
                        ppv_B = ps_pv.tile([128, 512], F32, tag="pv")
                        for sc in range(n_sc):
                            k = sc - 4 * tj  # >=0 on the causal diagonal
                            off = 128 * k if k > 0 else 0
                            pss = ps_s.tile([128, 1024], F32, tag="s")
                            # QK^T for both heads of the pair, row-packed
                            for hi, (half, ppos) in enumerate(
                                [(0, (0, 0)), (512, (64, 0))]
                            ):
                                prow = slice(64 * hi, 64 * hi + 64)
                                nc.tensor.matmul(
                                    pss[:, ds(half, 512)],
                                    kT[prow, hp, ts(sc, 128)],
                                    qT[prow, hp, ts(tj, 512)],
                                    start=True,
                                    stop=True,
                                    tile_position=ppos,
                                )
                                if k >= 0:
                                    # causal mask on the diagonal 128-block:
                                    # add -1e30 below the diagonal before exp
                                    nc.vector.tensor_tensor(
                                        pss[:, ds(half + off, 128)],
                                        pss[:, ds(half + off, 128)],
                                        atrif[:],
                                        ALU.add,
                                    )
                            psb = pexp.tile([128, 1024], F32R, tag="p")
                            nc.scalar.activation(psb[:], pss[:], AF.Exp)
                            # fully-masked columns [0, off) are skipped (their
                            # psb values are garbage but never read)
                            for hi, h in ((0, hA), (1, hB)):
                                ppv = ppv_A if hi == 0 else ppv_B
                                nc.tensor.matmul(
                                    ppv[: DH + 1, ds(off, 512 - off)],
                                    vA[:, sc, ds(h * (DH + 1), DH + 1)],
                                    psb[:, ds(512 * hi + off, 512 - off)],
                                    start=(sc == 0),
                                    stop=(sc == n_sc - 1),
                                )
                        # normalize: y^T = Y_unnorm^T * (1/denom)
                        for hi, h in ((0, hA), (1, hB)):
                            ppv = ppv_A if hi == 0 else ppv_B
                            den = psm.tile([1, 512], F32, tag="den")
                            nc.vector.tensor_copy(den[:], ppv[DH : DH + 1, :])
                            rec = psm.tile([1, 512], F32, tag="rec")
                            scr = psm.tile([1, 512], F32, tag="scr")
                            nc.vector.reciprocal_approx_accurate(
                                rec[:], den[:], scr[:]
                            )
                            recB = psm.tile([DH, 512], F32, tag="recB")
                            nc.gpsimd.partition_broadcast(recB[:], rec[:])
                            nc.vector.tensor_tensor(
                                yT[ds(64 * (h % 2), DH), h // 2, ts(tj, 512)],
                                ppv[:DH, :],
                                recB[:],
                                ALU.mult,
                            )

                    # o-projection for this t-chunk
                    for tt in range(4):
                        t0 = 512 * tj + 128 * tt
                        ot = pout.tile([128, C], F32, tag="o")
                        for nb in range(2):
                            po = ps_misc.tile([128, 512], F32, tag="misc")
                            for kk in range(2):
                                nc.tensor.matmul(
                                    po[:],
                                    yT[:, kk, ds(t0, 128)],
                                    wo_sb[:, kk, ts(nb, 512)],
                                    start=(kk == 0),
                                    stop=(kk == 1),
                                )
                            nc.vector.tensor_copy(ot[:, ts(nb, 512)], po[:])
                        nc.sync.dma_start(out.ap()[ds(t0, 128), :], ot[:])

    nc.compile()
    return nc


_CACHE = {}


def _get_program():
    if "nc" not in _CACHE:
        _CACHE["nc"] = build_program()
    return _CACHE["nc"]


def make_in_maps(x, wq, bq, wk, bk, wv, bv, wo):
    in_maps = []
    for core in range(N_CORES):
        b, g = core // 4, core % 4
        sl = slice(g * DQC, (g + 1) * DQC)
        in_maps.append(
            {
                "xb": np.ascontiguousarray(x[b]),
                "wq": np.ascontiguousarray(wq[:, sl]),
                "wk": np.ascontiguousarray(wk[:, sl]),
                "wv": np.ascontiguousarray(wv[:, sl]),
                "wo": np.ascontiguousarray(wo[sl, :]),
                "bq": np.ascontiguousarray(bq[sl]),
                "bk": np.ascontiguousarray(bk[sl]),
                "bv": np.ascontiguousarray(bv[sl]),
            }
        )
    return in_maps


def kernel(x, wq, bq, wk, bk, wv, bv, wo, bo):
    from concourse import bass_utils

    x = np.asarray(x, dtype=np.float32)
    wq = np.asarray(wq, dtype=np.float32)
    wk = np.asarray(wk, dtype=np.float32)
    wv = np.asarray(wv, dtype=np.float32)
    wo = np.asarray(wo, dtype=np.float32)
    bq = np.asarray(bq, dtype=np.float32)
    bk = np.asarray(bk, dtype=np.float32)
    bv = np.asarray(bv, dtype=np.float32)
    bo = np.asarray(bo, dtype=np.float32)

    nc = _get_program()
    in_maps = make_in_maps(x, wq, bq, wk, bk, wv, bv, wo)
    res = bass_utils.run_bass_kernel_spmd(
        nc, in_maps, core_ids=list(range(N_CORES))
    )
    y = np.zeros((B, T, C), dtype=np.float32)
    for core in range(N_CORES):
        y[core // 4] += res.results[core]["out"]
    y += bo
    return y



# revision 2
# speedup vs baseline: 1.3796x; 1.3796x over previous
"""Multi-head masked attention on 8 Trainium2 NeuronCores.

Sharding: data-parallel over batch (B=2 -> 2 groups of 4 cores),
tensor-parallel over heads within a group (16 heads -> 4 heads/core).
Each core computes q/k/v projections for its 4 heads (column-sharded),
causal flash-style attention in the transposed (S^T) domain, and a
row-sharded partial o-projection. The host sums the 4 partials per
batch element and adds the output bias.

Perf notes vs v1:
- x arrives pre-transposed ([C,T]) and in bf16 from the host, so the
  PE transpose phase and its psum->sbuf copies are gone and input DMA
  bytes are halved.
- All matmul operands are bf16 (psum stays f32), which keeps every
  matmul at 1 cycle/row regardless of moving size and halves LDWEIGHTS
  and SBUF traffic.
- Projections and o-projections are software-pipelined into the
  attention inner loop as "filler" PE work so the tensor engine never
  idles waiting on the scalar-engine exp (idle PE drops to a lower
  p-state and doubles matmul latency).
- Input DMAs are split across both hardware DGE queues (sync + act).

Self-contained: hardcodes shapes B=2, T=2048, C=1024, H=16, Dh=64.
"""

import sys

sys.path.insert(0, "/opt/trn_rl_repo")

import numpy as np

import concourse.bass as bass
import concourse.tile as tile
import concourse.mybir as mybir
from concourse import bacc
from concourse.bass import ts, ds
from concourse.masks import make_lower_triangular

F32 = mybir.dt.float32
BF16 = mybir.dt.bfloat16
AF = mybir.ActivationFunctionType
ALU = mybir.AluOpType

B, T, C = 2, 2048, 1024
H, DH = 16, 64
HPC = 4            # heads per core
DQC = HPC * DH     # 256 projected dims per core
N_CORES = 8
NEG = -1.0e30

TC = T // 128      # 16 s-chunks of 128
CC = C // 128      # 8 c-chunks
TJ = T // 512      # 4 t-chunks of 512
SCALE = 1.0 / np.sqrt(DH)


def build_program():
    nc = bacc.Bacc("TRN2", target_bir_lowering=False, debug=False)

    xt = nc.dram_tensor("xt", [C, T], BF16, kind="ExternalInput")
    wq = nc.dram_tensor("wq", [C, DQC], BF16, kind="ExternalInput")
    wk = nc.dram_tensor("wk", [C, DQC], BF16, kind="ExternalInput")
    wv = nc.dram_tensor("wv", [C, DQC], BF16, kind="ExternalInput")
    wo = nc.dram_tensor("wo", [DQC, C], BF16, kind="ExternalInput")
    bq = nc.dram_tensor("bq", [DQC], F32, kind="ExternalInput")
    bk = nc.dram_tensor("bk", [DQC], F32, kind="ExternalInput")
    bv = nc.dram_tensor("bv", [DQC], F32, kind="ExternalInput")
    out = nc.dram_tensor("out", [T, C], BF16, kind="ExternalOutput")

    with tile.TileContext(nc) as tc:
        with (
            tc.tile_pool(name="persist", bufs=1) as pp,
            tc.tile_pool(name="ps", bufs=1, space="PSUM") as ps,
            tc.tile_pool(name="psb_pool", bufs=3) as pexp,
            tc.tile_pool(name="small", bufs=4) as psm,
            tc.tile_pool(name="outp", bufs=3) as pout,
        ):
            # ---- persistent sbuf tensors -------------------------------
            xT = pp.tile([128, CC, T], BF16, tag="xT")
            wq_sb = pp.tile([128, CC, DQC], BF16, tag="wq")
            wk_sb = pp.tile([128, CC, DQC], BF16, tag="wk")
            wv_sb = pp.tile([128, CC, DQC], BF16, tag="wv")
            wo_sb = pp.tile([128, 2, C], BF16, tag="wo")
            qT = pp.tile([128, 2, T], BF16, tag="qT")   # [p=dq, hp, t]
            kT = pp.tile([128, 2, T], BF16, tag="kT")
            vA = pp.tile([128, TC, HPC * (DH + 1)], BF16, tag="vA")
            yT = pp.tile([128, 2, T], BF16, tag="yT")
            atrif = pp.tile([128, 128], F32, tag="atrif")
            bqs = pp.tile([128, 2], F32, tag="bqs")
            bks = pp.tile([128, 2], F32, tag="bks")
            bvs = pp.tile([128, DQC], F32, tag="bvs")

            # atrif[p, f] = NEG where f < p (mask s>t inside a diag block)
            make_lower_triangular(nc, atrif[:], val=NEG, diag=False)
            # ones column of v_aug (denominator trick)
            vA4 = vA[:].rearrange("p s (h d) -> p s h d", d=DH + 1)
            onesf = pp.tile([128, TC * HPC], F32, tag="onesf")
            nc.gpsimd.memset(onesf[:], 1.0)
            nc.vector.tensor_copy(
                vA4[:, :, :, DH : DH + 1],
                onesf[:].rearrange("p (s h o) -> p s h o", h=HPC, o=1),
            )

            # ---- input DMAs: x chunks on sync queue, weights on act ----
            xt_r = xt.ap().rearrange("(cc p) t -> p cc t", p=128)
            for tj in range(TJ):
                nc.sync.dma_start(xT[:, :, ts(tj, 512)], xt_r[:, :, ts(tj, 512)])

            nc.scalar.dma_start(bqs[:], bq.ap().rearrange("(k p) -> p k", p=128))
            nc.vector.tensor_scalar_mul(bqs[:], bqs[:], SCALE)
            nc.scalar.dma_start(bks[:], bk.ap().rearrange("(k p) -> p k", p=128))
            nc.scalar.dma_start(bvs[0:1, :], bv.ap().rearrange("(o n) -> o n", o=1))
            nc.gpsimd.partition_broadcast(bvs[:], bvs[0:1, :])
            nc.scalar.dma_start(wq_sb[:], wq.ap().rearrange("(c p) d -> p c d", p=128))
            nc.scalar.dma_start(wk_sb[:], wk.ap().rearrange("(c p) d -> p c d", p=128))
            nc.scalar.dma_start(wv_sb[:], wv.ap().rearrange("(c p) d -> p c d", p=128))
            nc.scalar.dma_start(wo_sb[:], wo.ap().rearrange("(k p) n -> p k n", p=128))

            # ---- PE work generators (software pipelining) --------------
            def gen_qkv_proj(tj):
                """Q^T/K^T projections for t-block tj + V for its 4 s-chunks.
                Yields after each matmul so the driver can interleave."""
                for hp in range(2):
                    pq = ps.tile([128, 512], F32, tag="po", bufs=2, name="pq")
                    for cc in range(CC):
                        nc.tensor.matmul(
                            pq[:],
                            wq_sb[:, cc, ts(hp, 128)],
                            xT[:, cc, ts(tj, 512)],
                            start=(cc == 0),
                            stop=(cc == CC - 1),
                        )
                        yield
                    nc.vector.tensor_scalar(
                        qT[:, hp, ts(tj, 512)],
                        pq[:],
                        SCALE,
                        bqs[:, hp : hp + 1],
                        ALU.mult,
                        ALU.add,
                    )
                    pk = ps.tile([128, 512], F32, tag="po", bufs=2, name="pk")
                    for cc in range(CC):
                        nc.tensor.matmul(
                            pk[:],
                            wk_sb[:, cc, ts(hp, 128)],
                            xT[:, cc, ts(tj, 512)],
                            start=(cc == 0),
                            stop=(cc == CC - 1),
                        )
                        yield
                    nc.vector.tensor_scalar(
                        kT[:, hp, ts(tj, 512)],
                        pk[:],
                        bks[:, hp : hp + 1],
                        None,
                        ALU.add,
                    )
                for scp in range(2):  # two pairs of s-chunks
                    pv = ps.tile([128, 512], F32, tag="po", bufs=2, name="pv")
                    for half in range(2):
                        sc = 4 * tj + 2 * scp + half
                        for cc in range(CC):
                            nc.tensor.matmul(
                                pv[:, ds(256 * half, 256)],
                                xT[:, cc, ts(sc, 128)],
                                wv_sb[:, cc, :],
                                start=(cc == 0),
                                stop=(cc == CC - 1),
                            )
                            yield
                    for half in range(2):
                        sc = 4 * tj + 2 * scp + half
                        nc.vector.tensor_tensor(
                            vA4[:, sc, :, :DH],
                            pv[:, ds(256 * half, 256)].rearrange(
                                "p (h d) -> p h d", d=DH
                            ),
                            bvs[:].rearrange("p (h d) -> p h d", d=DH),
                            ALU.add,
                        )
                    yield

            def gen_oproj(tj):
                """o-projection of t-block tj (partial, row-sharded)."""
                for tt in range(4):
                    t0 = 512 * tj + 128 * tt
                    ot = pout.tile([128, C], BF16, tag="o", name="ot")
                    for nb in range(2):
                        po = ps.tile([128, 512], F32, tag="po", bufs=2, name="po")
                        for kk in range(2):
                            nc.tensor.matmul(
                                po[:],
                                yT[:, kk, ds(t0, 128)],
                                wo_sb[:, kk, ts(nb, 512)],
                                start=(kk == 0),
                                stop=(kk == 1),
                            )
                            yield
                        nc.vector.tensor_copy(ot[:, ts(nb, 512)], po[:])
                    eng = nc.sync if tt % 2 == 0 else nc.scalar
                    eng.dma_start(out.ap()[ds(t0, 128), :], ot[:])
                    yield

            fillers = []

            def pump(n):
                done = 0
                while fillers and done < n:
                    try:
                        next(fillers[0])
                        done += 1
                    except StopIteration:
                        fillers.pop(0)

            def drain():
                pump(1 << 30)

            # ---- main pipeline -----------------------------------------
            fillers.append(gen_qkv_proj(0))
            drain()

            for tj in range(TJ):
                # work to interleave into this t-block's attention
                if tj + 1 < TJ:
                    fillers.append(gen_qkv_proj(tj + 1))
                if tj > 0:
                    fillers.append(gen_oproj(tj - 1))

                n_sc = 4 * (tj + 1)
                for hp in range(2):
                    hA, hB = 2 * hp, 2 * hp + 1
                    ppv_A = ps.tile([128, 512], F32, tag="pv0", bufs=1, name="ppvA")
                    ppv_B = ps.tile([128, 512], F32, tag="pv1", bufs=1, name="ppvB")
                    for sc in range(n_sc):
                        pump(2)
                        k = sc - 4 * tj  # >=0 on the causal diagonal
                        off = 128 * k if k > 0 else 0
                        pss = ps.tile([128, 1024], F32, tag="s", bufs=2, name="pss")
                        # QK^T for both heads of the pair, quadrant-packed
                        for hi, (half, ppos) in enumerate(
                            [(0, (0, 0)), (512, (64, 0))]
                        ):
                            prow = slice(64 * hi, 64 * hi + 64)
                            nc.tensor.matmul(
                                pss[:, ds(half, 512)],
                                kT[prow, hp, ts(sc, 128)],
                                qT[prow, hp, ts(tj, 512)],
                                start=True,
                                stop=True,
                                tile_position=ppos,
                            )
                            if k >= 0:
                                # causal mask on the diagonal 128-block
                                nc.vector.tensor_tensor(
                                    pss[:, ds(half + off, 128)],
                                    pss[:, ds(half + off, 128)],
                                    atrif[:],
                                    ALU.add,
                                )
                        psb = pexp.tile([128, 1024], BF16, tag="p", name="psb")
                        nc.scalar.activation(psb[:], pss[:], AF.Exp)
                        # fully-masked columns [0, off) are skipped (their
                        # psb values are garbage but never read)
                        for hi, h in ((0, hA), (1, hB)):
                            ppv = ppv_A if hi == 0 else ppv_B
                            nc.tensor.matmul(
                                ppv[: DH + 1, ds(off, 512 - off)],
                                vA[:, sc, ds(h * (DH + 1), DH + 1)],
                                psb[:, ds(512 * hi + off, 512 - off)],
                                start=(sc == 0),
                                stop=(sc == n_sc - 1),
                            )
                    # normalize: y^T = Y_unnorm^T * (1/denom)
                    for hi, h in ((0, hA), (1, hB)):
                        ppv = ppv_A if hi == 0 else ppv_B
                        den = psm.tile([1, 512], F32, tag="den", name="den")
                        nc.vector.tensor_copy(den[:], ppv[DH : DH + 1, :])
                        rec = psm.tile([1, 512], F32, tag="rec", name="rec")
                        scr = psm.tile([1, 512], F32, tag="scr", name="scr")
                        nc.vector.reciprocal_approx_accurate(rec[:], den[:], scr[:])
                        recB = psm.tile([DH, 512], F32, tag="recB", name="recB")
                        nc.gpsimd.partition_broadcast(recB[:], rec[:])
                        nc.vector.tensor_tensor(
                            yT[ds(64 * (h % 2), DH), h // 2, ts(tj, 512)],
                            ppv[:DH, :],
                            recB[:],
                            ALU.mult,
                        )
                drain()

            fillers.append(gen_oproj(TJ - 1))
            drain()

    nc.compile()
    return nc


_CACHE = {}


def _get_program():
    if "nc" not in _CACHE:
        _CACHE["nc"] = build_program()
    return _CACHE["nc"]


def make_in_maps(x, wq, bq, wk, bk, wv, bv, wo):
    import ml_dtypes

    bf16 = ml_dtypes.bfloat16
    xT_b = [np.ascontiguousarray(x[b].T).astype(bf16) for b in range(B)]
    in_maps = []
    for core in range(N_CORES):
        b, g = core // 4, core % 4
        sl = slice(g * DQC, (g + 1) * DQC)
        in_maps.append(
            {
                "xt": xT_b[b],
                "wq": np.ascontiguousarray(wq[:, sl]).astype(bf16),
                "wk": np.ascontiguousarray(wk[:, sl]).astype(bf16),
                "wv": np.ascontiguousarray(wv[:, sl]).astype(bf16),
                "wo": np.ascontiguousarray(wo[sl, :]).astype(bf16),
                "bq": np.ascontiguousarray(bq[sl]),
                "bk": np.ascontiguousarray(bk[sl]),
                "bv": np.ascontiguousarray(bv[sl]),
            }
        )
    return in_maps


def kernel(x, wq, bq, wk, bk, wv, bv, wo, bo):
    from concourse import bass_utils

    x = np.asarray(x, dtype=np.float32)
    wq = np.asarray(wq, dtype=np.float32)
    wk = np.asarray(wk, dtype=np.float32)
    wv = np.asarray(wv, dtype=np.float32)
    wo = np.asarray(wo, dtype=np.float32)
    bq = np.asarray(bq, dtype=np.float32)
    bk = np.asarray(bk, dtype=np.float32)
    bv = np.asarray(bv, dtype=np.float32)
    bo = np.asarray(bo, dtype=np.float32)

    nc = _get_program()
    in_maps = make_in_maps(x, wq, bq, wk, bk, wv, bv, wo)
    res = bass_utils.run_bass_kernel_spmd(
        nc, in_maps, core_ids=list(range(N_CORES))
    )
    y = np.zeros((B, T, C), dtype=np.float32)
    for core in range(N_CORES):
        y[core // 4] += np.asarray(res.results[core]["out"], dtype=np.float32)
    y += bo
    return y


# revision 14
# speedup vs baseline: 1.7005x; 1.2326x over previous
"""Multi-head masked attention on 8 Trainium2 NeuronCores.

Sharding: data-parallel over batch (B=2 -> 2 groups of 4 cores),
tensor-parallel over heads within a group (16 heads -> 4 heads/core).
Each core computes q/k/v projections for its 4 heads (column-sharded),
causal flash-style attention in the transposed (S^T) domain, and a
row-sharded partial o-projection. The host sums the 4 partials per
batch element and adds the output bias.

Perf structure:
- x arrives pre-transposed ([C,T]) and in bf16 from the host: no PE
  transposes, half the input DMA bytes.
- All matmul operands are bf16 (psum f32): 1 cycle/row for every
  moving size, small LDWEIGHTS.
- The causal mask is applied post-exp as a bf16 0/1 multiply in SBUF
  (DVE 2x mode) so the scalar-engine exp depends only on the QK psum.
- Projections and o-projections are split into fine-grained generators
  and interleaved into the attention loop as PE "filler" work: the
  tensor engine never idles (idle PE drops p-state and doubles matmul
  latency). V chunks and the second head-pair's Q/K are force-drained
  just before their first consumer.
- Initial DMAs are interleaved chunk-wise across both HWDGE queues in
  consumption order; no DMA issue lands on the act engine once exp
  work starts.

Self-contained: hardcodes shapes B=2, T=2048, C=1024, H=16, Dh=64.
"""

import sys

sys.path.insert(0, "/opt/trn_rl_repo")

import numpy as np

import concourse.bass as bass
import concourse.tile as tile
import concourse.mybir as mybir
from concourse import bacc
from concourse.bass import ts, ds
from concourse.masks import make_upper_triangular

F32 = mybir.dt.float32
BF16 = mybir.dt.bfloat16
AF = mybir.ActivationFunctionType
ALU = mybir.AluOpType

B, T, C = 2, 2048, 1024
H, DH = 16, 64
HPC = 4            # heads per core
DQC = HPC * DH     # 256 projected dims per core
N_CORES = 8
TC = T // 128      # 16 s-chunks of 128
CC = C // 128      # 8 c-chunks
TJ = T // 512      # 4 t-chunks of 512
SCALE = 1.0 / np.sqrt(DH)


def build_program():
    nc = bacc.Bacc("TRN2", target_bir_lowering=False, debug=False)

    xt = nc.dram_tensor("xt", [C, T], BF16, kind="ExternalInput")
    wq = nc.dram_tensor("wq", [C, DQC], BF16, kind="ExternalInput")
    wk = nc.dram_tensor("wk", [C, DQC], BF16, kind="ExternalInput")
    wv = nc.dram_tensor("wv", [C, DQC], BF16, kind="ExternalInput")
    wo = nc.dram_tensor("wo", [DQC, C], BF16, kind="ExternalInput")
    bq = nc.dram_tensor("bq", [DQC], F32, kind="ExternalInput")
    bk = nc.dram_tensor("bk", [DQC], F32, kind="ExternalInput")
    bv = nc.dram_tensor("bv", [DQC], F32, kind="ExternalInput")
    out = nc.dram_tensor("out", [T, C], BF16, kind="ExternalOutput")

    with tile.TileContext(nc) as tc:
        with (
            tc.tile_pool(name="persist", bufs=1) as pp,
            tc.tile_pool(name="ps", bufs=1, space="PSUM") as ps,
            tc.tile_pool(name="psb_pool", bufs=4) as pexp,
            tc.tile_pool(name="small", bufs=4) as psm,
            tc.tile_pool(name="outp", bufs=3) as pout,
        ):
            # ---- persistent sbuf tensors -------------------------------
            xT = pp.tile([128, CC, T], BF16, tag="xT")
            wq_sb = pp.tile([128, CC, DQC], BF16, tag="wq")
            wk_sb = pp.tile([128, CC, DQC], BF16, tag="wk")
            wv_sb = pp.tile([128, CC, DQC], BF16, tag="wv")
            wo_sb = pp.tile([128, 2, C], BF16, tag="wo")
            qT = pp.tile([128, 2, T], BF16, tag="qT")   # [p=dq, hp, t]
            kT = pp.tile([128, 2, T], BF16, tag="kT")
            vA = pp.tile([128, TC, HPC * (DH + 1)], BF16, tag="vA")
            yT = pp.tile([128, 2, T], BF16, tag="yT")
            trimask = pp.tile([128, 128], BF16, tag="trimask")
            bqs = pp.tile([128, 2], F32, tag="bqs")
            bks = pp.tile([128, 2], F32, tag="bks")
            bvs = pp.tile([128, DQC], F32, tag="bvs")

            # trimask[p, f] = 1 where f >= p else 0 (keep-mask for the
            # diagonal 128-block, applied post-exp)
            make_upper_triangular(nc, trimask[:], val=1.0, diag=True)
            # ones column of v_aug (denominator trick)
            vA4 = vA[:].rearrange("p s (h d) -> p s h d", d=DH + 1)
            onesf = pp.tile([128, TC * HPC], F32, tag="onesf")
            nc.gpsimd.memset(onesf[:], 1.0)
            nc.vector.tensor_copy(
                vA4[:, :, :, DH : DH + 1],
                onesf[:].rearrange("p (s h o) -> p s h o", h=HPC, o=1),
            )

            # ---- input DMAs: interleaved across both HWDGE queues in
            # consumption order ------------------------------------------
            xt_r = xt.ap().rearrange("(cc p) t -> p cc t", p=128)
            wq_r = wq.ap().rearrange("(c p) d -> p c d", p=128)
            wk_r = wk.ap().rearrange("(c p) d -> p c d", p=128)
            wv_r = wv.ap().rearrange("(c p) d -> p c d", p=128)
            wo_r = wo.ap().rearrange("(k p) n -> p k n", p=128)

            def q2(eng, a, b):
                eng.dma_start(a, b)

            # x t-block 0 chunks 0-3 then wq, alternating queues
            for cc in range(4):
                q2(nc.sync if cc % 2 == 0 else nc.scalar,
                   xT[:, cc, ts(0, 512)], xt_r[:, cc, ts(0, 512)])
            nc.sync.dma_start(wq_sb[:, :4, :], wq_r[:, :4, :])
            nc.scalar.dma_start(wq_sb[:, 4:, :], wq_r[:, 4:, :])
            for cc in range(4, CC):
                q2(nc.sync if cc % 2 == 0 else nc.scalar,
                   xT[:, cc, ts(0, 512)], xt_r[:, cc, ts(0, 512)])
            nc.sync.dma_start(wk_sb[:, :4, :], wk_r[:, :4, :])
            nc.scalar.dma_start(wk_sb[:, 4:, :], wk_r[:, 4:, :])
            nc.sync.dma_start(wv_sb[:, :4, :], wv_r[:, :4, :])
            nc.scalar.dma_start(wv_sb[:, 4:, :], wv_r[:, 4:, :])
            nc.scalar.dma_start(bqs[:], bq.ap().rearrange("(k p) -> p k", p=128))
            nc.vector.tensor_scalar_mul(bqs[:], bqs[:], SCALE)
            nc.scalar.dma_start(bks[:], bk.ap().rearrange("(k p) -> p k", p=128))
            nc.scalar.dma_start(bvs[0:1, :], bv.ap().rearrange("(o n) -> o n", o=1))
            nc.gpsimd.partition_broadcast(bvs[:], bvs[0:1, :])
            nc.sync.dma_start(xT[:, :4, ts(1, 512)], xt_r[:, :4, ts(1, 512)])
            nc.scalar.dma_start(xT[:, 4:, ts(1, 512)], xt_r[:, 4:, ts(1, 512)])
            nc.sync.dma_start(wo_sb[:, 0, :], wo_r[:, 0, :])
            nc.scalar.dma_start(wo_sb[:, 1, :], wo_r[:, 1, :])
            nc.sync.dma_start(xT[:, :4, ts(2, 512)], xt_r[:, :4, ts(2, 512)])
            nc.scalar.dma_start(xT[:, 4:, ts(2, 512)], xt_r[:, 4:, ts(2, 512)])
            nc.sync.dma_start(xT[:, :4, ts(3, 512)], xt_r[:, :4, ts(3, 512)])
            nc.scalar.dma_start(xT[:, 4:, ts(3, 512)], xt_r[:, 4:, ts(3, 512)])

            # ---- PE work generators (software pipelining) --------------
            def gen_qk_proj(tj, hp):
                """Q^T and K^T projection for (t-block tj, head-pair hp)."""
                pq = ps.tile([128, 512], F32, tag="po", bufs=2, name="pq")
                for cc in range(CC):
                    nc.tensor.matmul(
                        pq[:],
                        wq_sb[:, cc, ts(hp, 128)],
                        xT[:, cc, ts(tj, 512)],
                        start=(cc == 0),
                        stop=(cc == CC - 1),
                    )
                    yield
                nc.vector.tensor_scalar(
                    qT[:, hp, ts(tj, 512)],
                    pq[:],
                    SCALE,
                    bqs[:, hp : hp + 1],
                    ALU.mult,
                    ALU.add,
                )
                pk = ps.tile([128, 512], F32, tag="po", bufs=2, name="pk")
                for cc in range(CC):
                    nc.tensor.matmul(
                        pk[:],
                        wk_sb[:, cc, ts(hp, 128)],
                        xT[:, cc, ts(tj, 512)],
                        start=(cc == 0),
                        stop=(cc == CC - 1),
                    )
                    yield
                nc.vector.tensor_scalar(
                    kT[:, hp, ts(tj, 512)],
                    pk[:],
                    bks[:, hp : hp + 1],
                    None,
                    ALU.add,
                )
                yield

            def gen_v_pair(tj, scp):
                """V projection for s-chunks 4tj+2scp and 4tj+2scp+1."""
                pv = ps.tile([128, 512], F32, tag="po", bufs=2, name="pv")
                for half in range(2):
                    sc = 4 * tj + 2 * scp + half
                    for cc in range(CC):
                        nc.tensor.matmul(
                            pv[:, ds(256 * half, 256)],
                            xT[:, cc, ts(sc, 128)],
                            wv_sb[:, cc, :],
                            start=(cc == 0),
                            stop=(cc == CC - 1),
                        )
                        yield
                for half in range(2):
                    sc = 4 * tj + 2 * scp + half
                    nc.vector.tensor_tensor(
                        vA4[:, sc, :, :DH],
                        pv[:, ds(256 * half, 256)].rearrange(
                            "p (h d) -> p h d", d=DH
                        ),
                        bvs[:].rearrange("p (h d) -> p h d", d=DH),
                        ALU.add,
                    )
                yield

            def gen_oproj(tj):
                """o-projection of t-block tj (partial, row-sharded)."""
                for tt in range(4):
                    t0 = 512 * tj + 128 * tt
                    ot = pout.tile([128, C], BF16, tag="o", name="ot")
                    for nb in range(2):
                        po = ps.tile([128, 512], F32, tag="po", bufs=2, name="po")
                        for kk in range(2):
                            nc.tensor.matmul(
                                po[:],
                                yT[:, kk, ds(t0, 128)],
                                wo_sb[:, kk, ts(nb, 512)],
                                start=(kk == 0),
                                stop=(kk == 1),
                            )
                            yield
                        nc.vector.tensor_copy(ot[:, ts(nb, 512)], po[:])
                        nc.sync.dma_start(
                            out.ap()[ds(t0, 128), ts(nb, 512)], ot[:, ts(nb, 512)]
                        )
                    yield

            fillers = []

            def pump(n):
                done = 0
                while fillers and done < n:
                    try:
                        next(fillers[0])
                        done += 1
                    except StopIteration:
                        fillers.pop(0)

            def force(g):
                """Fully drain one generator (and remove from fillers)."""
                for _ in g:
                    pass
                if g in fillers:
                    fillers.remove(g)

            # ---- main pipeline -----------------------------------------
            qk0 = {0: gen_qk_proj(0, 0)}
            force(qk0[0])

            for tj in range(TJ):
                # (tj, hp=0) Q/K must be fully emitted before its attention
                force(qk0[tj])
                vpair = {scp: gen_v_pair(tj, scp) for scp in range(2)}
                qk1 = gen_qk_proj(tj, 1)
                fillers.insert(0, qk1)
                if tj > 0:
                    fillers.append(gen_oproj(tj - 1))
                if tj + 1 < TJ:
                    qk0[tj + 1] = gen_qk_proj(tj + 1, 0)
                    fillers.append(qk0[tj + 1])

                n_sc = 4 * (tj + 1)
                for hp in range(2):
                    if hp == 1:
                        force(qk1)
                    hA, hB = 2 * hp, 2 * hp + 1
                    ppv_A = ps.tile([128, 512], F32, tag="pv0", bufs=1, name="ppvA")
                    ppv_B = ps.tile([128, 512], F32, tag="pv1", bufs=1, name="ppvB")
                    for sc in range(n_sc):
                        k = sc - 4 * tj  # >=0 on the causal diagonal
                        if hp == 0 and k >= 0 and k % 2 == 0:
                            force(vpair[k // 2])
                        # extra fillers at group start cover the previous
                        # group's normalize latency (ppv banks busy)
                        pump(6 if sc == 0 else 2)
                        # skip leading fully-masked columns on diagonal blocks
                        off = 128 * k if k > 0 else 0
                        pss = ps.tile([128, 1024], F32, tag="s", bufs=2, name="pss")
                        # QK^T for both heads of the pair, quadrant-packed
                        for hi, (half, ppos) in enumerate(
                            [(0, (0, 0)), (512, (64, 0))]
                        ):
                            prow = slice(64 * hi, 64 * hi + 64)
                            nc.tensor.matmul(
                                pss[:, ds(half + off, 512 - off)],
                                kT[prow, hp, ts(sc, 128)],
                                qT[prow, hp, ds(512 * tj + off, 512 - off)],
                                start=True,
                                stop=True,
                                tile_position=ppos,
                            )
                        psb = pexp.tile([128, 1024], BF16, tag="p", name="psb")
                        if off == 0:
                            nc.scalar.activation(psb[:], pss[:], AF.Exp)
                        else:
                            # one instruction over both halves' live columns
                            nc.scalar.activation(
                                psb[:].rearrange("p (h t) -> p h t", t=512)[
                                    :, :, ds(off, 512 - off)
                                ],
                                pss[:].rearrange("p (h t) -> p h t", t=512)[
                                    :, :, ds(off, 512 - off)
                                ],
                                AF.Exp,
                            )
                        if k >= 0:
                            # causal keep-mask on the diagonal 128-block
                            for half in (0, 512):
                                nc.vector.tensor_tensor(
                                    psb[:, ds(half + off, 128)],
                                    psb[:, ds(half + off, 128)],
                                    trimask[:],
                                    ALU.mult,
                                )
                        for hi, h in ((0, hA), (1, hB)):
                            ppv = ppv_A if hi == 0 else ppv_B
                            nc.tensor.matmul(
                                ppv[: DH + 1, ds(off, 512 - off)],
                                vA[:, sc, ds(h * (DH + 1), DH + 1)],
                                psb[:, ds(512 * hi + off, 512 - off)],
                                start=(sc == 0),
                                stop=(sc == n_sc - 1),
                            )
                    # normalize: y^T = Y_unnorm^T * (1/denom)
                    for hi, h in ((0, hA), (1, hB)):
                        ppv = ppv_A if hi == 0 else ppv_B
                        den = psm.tile([1, 512], F32, tag="den", name="den")
                        nc.vector.tensor_copy(den[:], ppv[DH : DH + 1, :])
                        rec = psm.tile([1, 512], F32, tag="rec", name="rec")
                        nc.vector.reciprocal_approx_fast(rec[:], den[:])
                        recB = psm.tile([DH, 512], F32, tag="recB", name="recB")
                        nc.gpsimd.partition_broadcast(recB[:], rec[:])
                        nc.vector.tensor_tensor(
                            yT[ds(64 * (h % 2), DH), h // 2, ts(tj, 512)],
                            ppv[:DH, :],
                            recB[:],
                            ALU.mult,
                        )

            while fillers:
                force(fillers[0])
            force(gen_oproj(TJ - 1))

    nc.compile()
    return nc


_CACHE = {}


def _get_program():
    if "nc" not in _CACHE:
        _CACHE["nc"] = build_program()
    return _CACHE["nc"]


def make_in_maps(x, wq, bq, wk, bk, wv, bv, wo):
    import ml_dtypes

    bf16 = ml_dtypes.bfloat16
    xT_b = [np.ascontiguousarray(x[b].T).astype(bf16) for b in range(B)]
    in_maps = []
    for core in range(N_CORES):
        b, g = core // 4, core % 4
        sl = slice(g * DQC, (g + 1) * DQC)
        in_maps.append(
            {
                "xt": xT_b[b],
                "wq": np.ascontiguousarray(wq[:, sl]).astype(bf16),
                "wk": np.ascontiguousarray(wk[:, sl]).astype(bf16),
                "wv": np.ascontiguousarray(wv[:, sl]).astype(bf16),
                "wo": np.ascontiguousarray(wo[sl, :]).astype(bf16),
                "bq": np.ascontiguousarray(bq[sl]),
                "bk": np.ascontiguousarray(bk[sl]),
                "bv": np.ascontiguousarray(bv[sl]),
            }
        )
    return in_maps


def kernel(x, wq, bq, wk, bk, wv, bv, wo, bo):
    from concourse import bass_utils

    x = np.asarray(x, dtype=np.float32)
    wq = np.asarray(wq, dtype=np.float32)
    wk = np.asarray(wk, dtype=np.float32)
    wv = np.asarray(wv, dtype=np.float32)
    wo = np.asarray(wo, dtype=np.float32)
    bq = np.asarray(bq, dtype=np.float32)
    bk = np.asarray(bk, dtype=np.float32)
    bv = np.asarray(bv, dtype=np.float32)
    bo = np.asarray(bo, dtype=np.float32)

    nc = _get_program()
    in_maps = make_in_maps(x, wq, bq, wk, bk, wv, bv, wo)
    res = bass_utils.run_bass_kernel_spmd(
        nc, in_maps, core_ids=list(range(N_CORES))
    )
    y = np.zeros((B, T, C), dtype=np.float32)
    for core in range(N_CORES):
        y[core // 4] += np.asarray(res.results[core]["out"], dtype=np.float32)
    y += bo
    return y
